# revision 50
# baseline (speedup 1.0000x reference)
"""Trainium2 Bass kernel for Bahdanau (MLP) additive attention.

Reference computation (B=4, T=128, S=512, H=512):
    wq = dec @ Wq.T + bq                    [B,T,H]
    uh = enc @ Wc.T                         [B,S,H]
    scores[b,t,s] = sum_h v[h] * tanh(wq[b,t,h] + uh[b,s,h])
    align = softmax(scores, axis=-1)        [B,T,S]
    c = align @ enc                         [B,T,H]
    attn_h = [c, dec] @ Wo.T + bo           [B,T,H]
    returns (attn_h, align.transpose(1,0,2))

Sharding: pure data parallel over (batch, T-half) -> 8 cores, 64 queries
per core, no cross-core communication.  The dominant cost is tanh over
B*T*S*H = 134M elements: 16.8M per core on the scalar (ACT) engine at
1 elem/lane/cycle -> ~110 us busy, which this kernel keeps ~97% fed.

Per-core dataflow (hidden index on partitions everywhere):
    PE:  uhT = Wc.T-chunks @ encT, wqT = Wq.T-chunks @ decT (+bq), fp16
         inputs (fp16 rounding ~5e-4, 4x cheaper DMA + FWL weight loads)
    DVE: sum(t,hc) = uhT[hc] + wqT[hc, t]: tensor_scalar add with the
         per-partition scalar taken from wqT column t; fp16 in/out hits
         the DVE 4x perf mode (~270 ns per [128,512] slot)
    ACT: tanh over batched tiles (t-slots sharing one uh chunk), fp16
         out; batch width is 16 slots in interior phases (amortizing the
         352-cycle ACTIVATE overhead) and 8/4 at ramps and stripe ends
    PE:  scores[stripe] += V_window(t).T @ tanh(t,hc), f32r, N=512.
         V_window: a 32-wide sliding slice of a zero-padded buffer whose
         column t holds v_chunk - an M=32 matmul accumulates row t of the
         32-row stripe and adds zero to the other rows, working around
         the 32-aligned PE column-group constraint.
    Scores accumulate in two independent 32-row psum stripes so stripe
    0's softmax/output work overlaps stripe 1's main loop.
    softmax: exp straight off the scores psum (|scores| <= ||v||_1 ~ 21
         << 88, and softmax is shift-invariant, so no max subtraction)
         with fused row sums (accum_out), then DVE reciprocal + scale.
    out projection, prefolded in the prologue (PE, fp16):
         WET[s,k]  = sum_h enc[s,h] Wo[k,h]
         base[k,t] = sum_h Wo[k,H+h] dec[t,h] + bo  (pre-accumulated into
         the attn psum tiles via an identity matmul during the main loop)
    epilogue per stripe: alignT = transpose(align) on PE, then
         attn_T[k,t] = base + sum_s WET[s,k] alignT[s,t], copied out.

Further scheduling tricks: input DMAs split across the three DMA queues
(sync/scalar HWDGE + gpsimd SWDGE); PE warmed up with dummy matmuls so
the prologue matmuls run at 2.4 GHz; uh/wq chunks and the prefold are
emitted between loop phases so their PE/DVE work fills idle slots; the
first/last tanh batches are halved to shorten the pipeline ramps.

Measured on trn2 (8 cores): ~145 us HW exec, rel err ~7e-4 vs the fp32
jax reference.
"""

import numpy as np

B, T, S, H = 4, 128, 512, 512
P = 128
NH = H // P          # 4 h-chunks
NS = S // P          # 4 s-chunks
TC = 64              # queries per core
GB = 8               # t-slots per ACT batch (one uh chunk shared)
FD = GB * S          # 4096 free dim of the batched tanh tile
NG = 32 // GB        # groups per (stripe, hc) phase
N_CORES = 8

SCORES_MODE = "f32r"
SUMS_BF16 = True

_cached = None


def _build():
    import concourse.bacc as bacc
    import concourse.tile as tile
    import concourse.mybir as mybir
    from concourse.masks import make_identity

    f32 = mybir.dt.float32
    bf16 = mybir.dt.float16  # fp16: same speed, 4x finer rounding than bfloat16
    f32r = mybir.dt.float32r
    AF = mybir.ActivationFunctionType

    nc = bacc.Bacc("TRN2", target_bir_lowering=False, debug=False,
                   num_devices=N_CORES)

    d_decT = nc.dram_tensor("decT", [P, NH * TC], bf16, kind="ExternalInput")
    d_wqt = nc.dram_tensor("wqt", [P, NH * H], bf16, kind="ExternalInput")
    d_wct = nc.dram_tensor("wct", [P, NH * H], bf16, kind="ExternalInput")
    d_encT = nc.dram_tensor("encT", [P, NH * S], bf16, kind="ExternalInput")
    d_V = nc.dram_tensor("V", [P, NH * 64], bf16, kind="ExternalInput")
    d_bqb = nc.dram_tensor("bqb", [P, NH], f32, kind="ExternalInput")
    d_bob = nc.dram_tensor("bob", [P, NH], f32, kind="ExternalInput")
    d_wot = nc.dram_tensor("wot", [P, 2 * NH * H], bf16, kind="ExternalInput")

    d_align = nc.dram_tensor("align_out", [TC, S], f32, kind="ExternalOutput")
    d_attn = nc.dram_tensor("attn_out", [P, NH * TC], f32, kind="ExternalOutput")

    with tile.TileContext(nc) as tc:
        with (
            tc.tile_pool(name="consts", bufs=1) as consts,
            tc.tile_pool(name="sums", bufs=4) as sums,
            tc.tile_pool(name="tanhs", bufs=4) as tanhs,
            tc.tile_pool(name="small", bufs=4) as small,
            tc.tile_pool(name="psc", bufs=1, space="PSUM") as psum_sc,
            tc.tile_pool(name="pbig", bufs=2, space="PSUM") as psum_big,
            tc.tile_pool(name="psm", bufs=2, space="PSUM") as psum_sm,
        ):
            # ---- input DMAs, critical-path tensors first ----
            ident = consts.tile([32, 32], f32)
            make_identity(nc, ident[:, :])
            identB = consts.tile([P, P], bf16)
            make_identity(nc, identB[:, :])
            junk = consts.tile([P, S], bf16)
            nc.vector.memset(junk[:], 0.5)
            ps_warm = psum_sm.tile([P, S], f32, tag="sm", name="ps_warm")
            warm_insts = []
            for i in range(20):
                warm_insts.append(nc.tensor.matmul(
                    ps_warm[:], junk[:, :P], junk[:],
                    start=(i == 0), stop=(i == 19), skip_group_check=True))

            # three parallel DMA queues (sync/scalar HWDGE + gpsimd SWDGE),
            # each ~110 GB/s; the uh/wq inputs are split so every queue
            # carries ~0.5 MB of the critical path
            sb_encT = consts.tile([P, NH * S], bf16)
            sb_wct = consts.tile([P, NH * H], bf16)
            nc.sync.dma_start(sb_encT[:], d_encT[:])
            nc.scalar.dma_start(sb_wct[:], d_wct[:])
            sb_wqt = consts.tile([P, NH * H], bf16)
            nc.gpsimd.dma_start(sb_wqt[:], d_wqt[:])
            sb_decT = consts.tile([P, NH * TC], bf16)
            nc.sync.dma_start(sb_decT[:], d_decT[:])
            sb_V = consts.tile([P, NH * 64], bf16)
            nc.sync.dma_start(sb_V[:], d_V[:])
            sb_bqb = consts.tile([P, NH], f32)
            nc.sync.dma_start(sb_bqb[:], d_bqb[:])
            sb_bob = consts.tile([P, NH], f32)
            nc.scalar.dma_start(sb_bob[:], d_bob[:])
            sb_wot = consts.tile([P, 2 * NH * H], bf16)
            nc.sync.dma_start(sb_wot[:], d_wot[:])
            sb_encTb = sb_encT
            sb_decTb = sb_decT



            # ---- uhT[k, s] + wqT[k, t] (+bq), chunk-interleaved so the
            # first tanh batch only waits for the kc=0 chunks ----
            sum_dt = bf16 if SUMS_BF16 else f32
            sb_uh = consts.tile([P, NH * S], sum_dt)
            sb_wqb = consts.tile([P, NH * TC], f32)
            from concourse.tile import add_dep_helper

            def do_proj(kcs):
                for kc in kcs:
                    ps = psum_big.tile([P, S], f32, tag="big", name="psb")
                    for hc in range(NH):
                        mm = nc.tensor.matmul(
                            ps[:],
                            sb_wct[:, hc * H + kc * P: hc * H + (kc + 1) * P],
                            sb_encT[:, hc * S:(hc + 1) * S],
                            start=(hc == 0), stop=(hc == NH - 1),
                        )
                        if kc == 0 and hc == 0:
                            add_dep_helper(
                                mm.ins, warm_insts[-1].ins, sync=False,
                                reason="warmup before first uh matmul")
                    nc.vector.tensor_copy(sb_uh[:, kc * S:(kc + 1) * S], ps[:])
                    psq = psum_sm.tile([P, TC], f32, tag="sm", name="ps")
                    for hc in range(NH):
                        nc.tensor.matmul(
                            psq[:],
                            sb_wqt[:, hc * H + kc * P: hc * H + (kc + 1) * P],
                            sb_decT[:, hc * TC:(hc + 1) * TC],
                            start=(hc == 0), stop=(hc == NH - 1),
                        )
                    nc.vector.tensor_scalar_add(
                        sb_wqb[:, kc * TC:(kc + 1) * TC], psq[:],
                        sb_bqb[:, kc:kc + 1])

            do_proj([0])

            ps_scores = [
                psum_sc.tile([32, S], f32, tag="scores0", name="scores0"),
                psum_sc.tile([32, S], f32, tag="scores1", name="scores1"),
            ]

            def do_stripe(half, phases=range(NH)):
                h0 = half * 32
                for hc in phases:
                    # batch sizes tuned per phase: wide (16) batches in
                    # the interior phases amortize the per-ACTIVATE
                    # overhead; narrow (8/4) batches at the ramp-up and
                    # the stripe end keep the pipeline fine-grained where
                    # it matters
                    if hc == NH - 1:
                        batches = [(0, 16), (16, 8), (24, 4), (28, 4)]
                    elif hc == 0 and half == 0:
                        batches = [(0, 4), (4, 4), (8, 8), (16, 16)]
                    else:
                        batches = [(0, 16), (16, 16)]
                    for bt0, bn in batches:
                        sum_t = sums.tile([P, bn * S], sum_dt, tag="sum",
                                          name="sum_t")
                        for tt in range(bn):
                            t = h0 + bt0 + tt
                            nc.vector.tensor_scalar_add(
                                sum_t[:, tt * S:(tt + 1) * S],
                                sb_uh[:, hc * S:(hc + 1) * S],
                                sb_wqb[:, hc * TC + t: hc * TC + t + 1])
                        tanh_t = tanhs.tile([P, bn * S], bf16, tag="tanh",
                                            name="tanh_t")
                        nc.scalar.activation(tanh_t[:], sum_t[:], AF.Tanh)
                        for tt in range(bn):
                            t = h0 + bt0 + tt
                            tl = t - h0
                            lhsT = sb_V[:, hc * 64 + 32 - tl: hc * 64 + 64 - tl]
                            rhs = tanh_t[:, tt * S:(tt + 1) * S]
                            nc.tensor.matmul(
                                ps_scores[half][:, :], lhsT, rhs,
                                start=(hc == 0 and bt0 == 0 and tt == 0),
                                stop=(hc == NH - 1 and bt0 + tt == 32 - 1),
                                skip_group_check=True,
                            )

            # ---- prefolded output projection (emitted after stripe 0 so
            # its DVE work lands once the loop has built up a cushion) ----
            sb_WET = consts.tile([P, NS * H], bf16)
            sb_base = consts.tile([P, NH * TC], bf16)

            def do_prefold_wet(scs):
                # WET[s, k] = sum_h enc[s, h] * Wo[k, h]
                for sc in scs:
                    ps = psum_big.tile([P, H], f32, tag="big", name="psw")
                    for hc in range(NH):
                        nc.tensor.matmul(
                            ps[:],
                            sb_encTb[:, hc * S + sc * P: hc * S + (sc + 1) * P],
                            sb_wot[:, hc * H:(hc + 1) * H],
                            start=(hc == 0), stop=(hc == NH - 1),
                        )
                    nc.vector.tensor_copy(sb_WET[:, sc * H:(sc + 1) * H], ps[:])

            def do_prefold_base():
                # base[k, t] = sum_h Wo[k, H + h] * dec[t, h] + bo[k]
                for kc in range(NH):
                    ps = psum_sm.tile([P, TC], f32, tag="sm", name="psd")
                    for dc in range(NH, 2 * NH):
                        nc.tensor.matmul(
                            ps[:],
                            sb_wot[:, dc * H + kc * P: dc * H + (kc + 1) * P],
                            sb_decTb[:, (dc - NH) * TC:(dc - NH + 1) * TC],
                            start=(dc == NH), stop=(dc == 2 * NH - 1),
                        )
                    nc.vector.tensor_scalar_add(
                        sb_base[:, kc * TC:(kc + 1) * TC], ps[:],
                        sb_bob[:, kc:kc + 1])

            sb_aligns = [consts.tile([32, S], f32, name="sb_exp0"),
                         consts.tile([32, S], f32, name="sb_exp1")]
            sb_align_n = [consts.tile([32, S], f32, name="sb_al0"),
                          consts.tile([32, S], f32, name="sb_al1")]
            sb_alignT = consts.tile([P, NS * TC], bf16)
            sb_attn = consts.tile([P, NH * TC], f32)
            d_attn3 = d_attn[:].rearrange("p (k t) -> p k t", k=NH)
            sb_attn3 = sb_attn[:].rearrange("p (k t) -> p k t", k=NH)

            def do_epilogue(half):
                h0 = half * 32
                sb_align = sb_aligns[half]
                psc = ps_scores[half]
                # softmax pieces: exp with -max bias, row sums, 1/sum
                # no max-subtraction needed: |scores| <= ||v||_1 * 1 ~ 21,
                # far below fp32 exp overflow (~88), and softmax is
                # shift-invariant so the result matches the reference
                sumexp = small.tile([32, 1], f32, tag="st", name="sumexp")
                rsum = small.tile([32, 1], f32, tag="st", name="rsum")
                nc.scalar.activation(sb_align[:, :], psc[:, :], AF.Exp,
                                     accum_out=sumexp[:])
                nc.vector.reciprocal(rsum[:], sumexp[:])
                # normalized align rows for the align_vectors output
                nc.vector.tensor_scalar_mul(sb_align_n[half][:, :],
                                            sb_align[:, :], rsum[:])
                nc.sync.dma_start(d_align[h0:h0 + 32, :], sb_align_n[half][:, :])

                # alignT[s, t-half] from the normalized align rows
                for sc in range(NS):
                    pst = psum_sm.tile([P, 32], f32, tag="sm", name="pst")
                    nc.tensor.transpose(
                        pst[:], sb_align_n[half][:, sc * P:(sc + 1) * P],
                        ident[:])
                    nc.vector.tensor_copy(
                        sb_alignT[:, sc * TC + h0: sc * TC + h0 + 32], pst[:])

                # attn_T[k, t] = base[k, t] + sum_s WET[s, k] * alignT[s, t]
                # sc-major accumulation: each transposed alignT chunk feeds
                # all four kc groups immediately (base was pre-accumulated
                # into the psum tiles during the main loop)
                ps_att = att_psums[half]
                for sc in range(NS):
                    for kc in range(NH):
                        nc.tensor.matmul(
                            ps_att[kc][:],
                            sb_WET[:, sc * H + kc * P: sc * H + (kc + 1) * P],
                            sb_alignT[:, sc * TC + h0: sc * TC + h0 + 32],
                            start=False, stop=(sc == NS - 1),
                            skip_group_check=True)
                for kc in range(NH):
                    nc.vector.tensor_copy(
                        sb_attn3[:, kc, h0:h0 + 32], ps_att[kc][:])
                nc.sync.dma_start(d_attn3[:, :, h0:h0 + 32],
                                  sb_attn3[:, :, h0:h0 + 32])

            do_proj([1])
            do_stripe(0, phases=[0])
            do_proj([2])
            do_stripe(0, phases=[1])
            do_proj([3])
            do_stripe(0, phases=[2, 3])
            do_stripe(1, phases=[0])
            do_prefold_wet([0, 1])
            do_stripe(1, phases=[1])
            do_prefold_wet([2, 3])
            do_prefold_base()
            att_psums = {}
            for half in (0, 1):
                h0 = half * 32
                tiles = []
                for kc in range(NH):
                    tag = "big" if kc < 2 else "ps2"
                    ps = psum_sm.tile([P, 32], f32, tag=tag,
                                      name=f"att{half}{kc}") if kc >= 2 else \
                        psum_big.tile([P, 32], f32, tag="big",
                                      name=f"att{half}{kc}")
                    nc.tensor.matmul(
                        ps[:], identB[:, :],
                        sb_base[:, kc * TC + h0: kc * TC + h0 + 32],
                        start=True, stop=False, skip_group_check=True)
                    tiles.append(ps)
                att_psums[half] = tiles
            do_epilogue(0)
            do_stripe(1, phases=[2, 3])
            do_epilogue(1)

    nc.compile()
    return nc


def _get_nc():
    global _cached
    if _cached is None:
        _cached = _build()
    return _cached


def _chunk_cols(a):
    """[n*128, C] -> [128, n*C] with row-chunk i at cols [i*C:(i+1)*C]."""
    n = a.shape[0] // P
    return np.ascontiguousarray(
        a.reshape(n, P, a.shape[1]).transpose(1, 0, 2).reshape(P, -1))


def make_in_maps(dec_output, enc_output, Wq, bq, Wc, v, Wo, bo):
    bf16 = np.float16

    wqt = _chunk_cols(np.ascontiguousarray(Wq.T)).astype(bf16)
    wct = _chunk_cols(np.ascontiguousarray(Wc.T)).astype(bf16)
    wot = _chunk_cols(np.ascontiguousarray(Wo.T)).astype(bf16)
    bqb = np.ascontiguousarray(bq.reshape(NH, P).T)
    bob = np.ascontiguousarray(bo.reshape(NH, P).T)
    # Sliding-window padded v: column (hc*64 + 32) holds v chunk hc; the
    # lhsT slice [hc*64 + 32 - tl : hc*64 + 64 - tl] puts v at window col tl
    V = np.zeros((P, NH, 64), dtype=np.float32)
    for hc in range(NH):
        V[:, hc, 32] = v[hc * P:(hc + 1) * P]
    V = np.ascontiguousarray(V.reshape(P, NH * 64)).astype(bf16)

    in_maps = []
    for c in range(N_CORES):
        b, th = c // 2, c % 2
        t0 = th * TC
        decT = _chunk_cols(
            np.ascontiguousarray(dec_output[b].T[:, t0:t0 + TC])).astype(bf16)
        encT_sb = _chunk_cols(
            np.ascontiguousarray(enc_output[b].T)).astype(bf16)
        in_maps.append({
            "decT": decT, "encT": encT_sb,
            "wqt": wqt, "wct": wct, "wot": wot,
            "V": V, "bqb": bqb, "bob": bob,
        })
    return in_maps


def assemble(results):
    attn_h = np.empty((B, T, H), dtype=np.float32)
    align_vectors = np.empty((T, B, S), dtype=np.float32)
    for c in range(N_CORES):
        b, th = c // 2, c % 2
        t0 = th * TC
        align_vectors[t0:t0 + TC, b, :] = results[c]["align_out"]
        a = results[c]["attn_out"].reshape(P, NH, TC)
        attn_h[b, t0:t0 + TC, :] = a.transpose(2, 1, 0).reshape(TC, H)
    return attn_h, align_vectors


def run(trace=False, **inputs):
    from concourse.bass_utils import run_bass_kernel_spmd

    args = {k: np.asarray(inputs[k], dtype=np.float32)
            for k in ("dec_output", "enc_output", "Wq", "bq", "Wc", "v",
                      "Wo", "bo")}
    nc = _get_nc()
    in_maps = make_in_maps(**args)
    if trace:
        try:
            from antenv.axon_hooks import set_axon_ntff_profile_hook
            from trn_agent_boot.trn_boot import _ntff_profile_via_ctypes
            set_axon_ntff_profile_hook(
                _ntff_profile_via_ctypes("/opt/axon/libaxon_pjrt.so"))
        except Exception:
            pass
    res = run_bass_kernel_spmd(nc, in_maps, core_ids=list(range(N_CORES)),
                               trace=trace)
    out = assemble(res.results)
    return out, res


def kernel(**inputs):
    out, _ = run(trace=False, **inputs)
    return out


# revision 51
# speedup vs baseline: 1.1967x; 1.1967x over previous
"""Trainium2 Bass kernel for Bahdanau (MLP) additive attention.

Reference computation (B=4, T=128, S=512, H=512):
    wq = dec @ Wq.T + bq                    [B,T,H]
    uh = enc @ Wc.T                         [B,S,H]
    scores[b,t,s] = sum_h v[h] * tanh(wq[b,t,h] + uh[b,s,h])
    align = softmax(scores, axis=-1)        [B,T,S]
    c = align @ enc                         [B,T,H]
    attn_h = [c, dec] @ Wo.T + bo           [B,T,H]
    returns (attn_h, align.transpose(1,0,2))

Sharding: pure data parallel over (batch, T-half) -> 8 cores, 64 queries
per core, no cross-core communication.  The dominant cost is tanh over
B*T*S*H = 134M elements: 16.8M per core on the scalar (ACT) engine at
1 elem/lane/cycle -> ~110 us busy, which this kernel keeps ~97% fed.

Per-core dataflow (hidden index on partitions everywhere):
    PE:  uhT = Wc.T-chunks @ encT, wqT = Wq.T-chunks @ decT (+bq), fp16
         inputs (fp16 rounding ~5e-4, 4x cheaper DMA + FWL weight loads)
    DVE: sum(t,hc) = uhT[hc] + wqT[hc, t]: tensor_scalar add with the
         per-partition scalar taken from wqT column t; fp16 in/out hits
         the DVE 4x perf mode (~270 ns per [128,512] slot)
    ACT: tanh over batched tiles (t-slots sharing one uh chunk), fp16
         out; batch width is 16 slots in interior phases (amortizing the
         352-cycle ACTIVATE overhead) and 8/4 at ramps and stripe ends
    PE:  scores[stripe] += V_window(t).T @ tanh(t,hc), f32r, N=512.
         V_window: a 32-wide sliding slice of a zero-padded buffer whose
         column t holds v_chunk - an M=32 matmul accumulates row t of the
         32-row stripe and adds zero to the other rows, working around
         the 32-aligned PE column-group constraint.
    Scores accumulate in two independent 32-row psum stripes so stripe
    0's softmax/output work overlaps stripe 1's main loop.
    softmax: exp straight off the scores psum (|scores| <= ||v||_1 ~ 21
         << 88, and softmax is shift-invariant, so no max subtraction)
         with fused row sums (accum_out), then DVE reciprocal + scale.
    out projection, prefolded in the prologue (PE, fp16):
         WET[s,k]  = sum_h enc[s,h] Wo[k,h]
         base[k,t] = sum_h Wo[k,H+h] dec[t,h] + bo  (pre-accumulated into
         the attn psum tiles via an identity matmul during the main loop)
    epilogue per stripe: alignT = transpose(align) on PE, then
         attn_T[k,t] = base + sum_s WET[s,k] alignT[s,t], copied out.

Further scheduling tricks: input DMAs split across the three DMA queues
(sync/scalar HWDGE + gpsimd SWDGE); PE warmed up with dummy matmuls so
the prologue matmuls run at 2.4 GHz; uh/wq chunks and the prefold are
emitted between loop phases so their PE/DVE work fills idle slots; the
first/last tanh batches are halved to shorten the pipeline ramps.

Measured on trn2 (8 cores): ~145 us HW exec, rel err ~7e-4 vs the fp32
jax reference.
"""

import numpy as np

B, T, S, H = 4, 128, 512, 512
P = 128
NH = H // P          # 4 h-chunks
NS = S // P          # 4 s-chunks
TC = 64              # queries per core
GB = 8               # t-slots per ACT batch (one uh chunk shared)
FD = GB * S          # 4096 free dim of the batched tanh tile
NG = 32 // GB        # groups per (stripe, hc) phase
N_CORES = 8

SCORES_MODE = "f32r"
SUMS_BF16 = True

_cached = None


def _build():
    import concourse.bacc as bacc
    import concourse.tile as tile
    import concourse.mybir as mybir
    from concourse.masks import make_identity

    f32 = mybir.dt.float32
    bf16 = mybir.dt.float16  # fp16: same speed, 4x finer rounding than bfloat16
    f32r = mybir.dt.float32r
    AF = mybir.ActivationFunctionType

    nc = bacc.Bacc("TRN2", target_bir_lowering=False, debug=False,
                   num_devices=N_CORES)

    d_decT = nc.dram_tensor("decT", [P, NH * TC], bf16, kind="ExternalInput")
    d_wqt = nc.dram_tensor("wqt", [P, NH * H], bf16, kind="ExternalInput")
    d_wct = nc.dram_tensor("wct", [P, NH * H], bf16, kind="ExternalInput")
    d_encT = nc.dram_tensor("encT", [P, NH * S], bf16, kind="ExternalInput")
    d_V = nc.dram_tensor("V", [P, NH * 64], bf16, kind="ExternalInput")
    d_bqb = nc.dram_tensor("bqb", [P, NH], f32, kind="ExternalInput")
    d_bob = nc.dram_tensor("bob", [P, NH], f32, kind="ExternalInput")
    d_wot = nc.dram_tensor("wot", [P, 2 * NH * H], bf16, kind="ExternalInput")

    d_align = nc.dram_tensor("align_out", [TC, S], f32, kind="ExternalOutput")
    d_attn = nc.dram_tensor("attn_out", [P, NH * TC], f32, kind="ExternalOutput")

    with tile.TileContext(nc) as tc:
        with (
            tc.tile_pool(name="consts", bufs=1) as consts,
            tc.tile_pool(name="sums", bufs=4) as sums,
            tc.tile_pool(name="tanhs", bufs=4) as tanhs,
            tc.tile_pool(name="small", bufs=4) as small,
            tc.tile_pool(name="psc", bufs=1, space="PSUM") as psum_sc,
            tc.tile_pool(name="pbig", bufs=2, space="PSUM") as psum_big,
            tc.tile_pool(name="psm", bufs=2, space="PSUM") as psum_sm,
        ):
            # ---- input DMAs, critical-path tensors first ----
            ident = consts.tile([32, 32], f32)
            make_identity(nc, ident[:, :])
            identB = consts.tile([P, P], bf16)
            make_identity(nc, identB[:, :])
            junk = consts.tile([P, S], bf16)
            nc.vector.memset(junk[:], 0.5)
            ps_warm = psum_sm.tile([P, S], f32, tag="sm", name="ps_warm")
            warm_insts = []
            for i in range(20):
                warm_insts.append(nc.tensor.matmul(
                    ps_warm[:], junk[:, :P], junk[:],
                    start=(i == 0), stop=(i == 19), skip_group_check=True))

            # three parallel DMA queues (sync/scalar HWDGE + gpsimd SWDGE),
            # each ~110 GB/s; the uh/wq inputs are split so every queue
            # carries ~0.5 MB of the critical path
            sb_encT = consts.tile([P, NH * S], bf16)
            sb_wct = consts.tile([P, NH * H], bf16)
            nc.sync.dma_start(sb_encT[:], d_encT[:])
            nc.scalar.dma_start(sb_wct[:], d_wct[:])
            sb_wqt = consts.tile([P, NH * H], bf16)
            nc.gpsimd.dma_start(sb_wqt[:], d_wqt[:])
            sb_decT = consts.tile([P, NH * TC], bf16)
            nc.sync.dma_start(sb_decT[:], d_decT[:])
            sb_V = consts.tile([P, NH * 64], bf16)
            nc.sync.dma_start(sb_V[:], d_V[:])
            sb_bqb = consts.tile([P, NH], f32)
            nc.sync.dma_start(sb_bqb[:], d_bqb[:])
            sb_bob = consts.tile([P, NH], f32)
            nc.scalar.dma_start(sb_bob[:], d_bob[:])
            sb_wot = consts.tile([P, 2 * NH * H], bf16)
            nc.sync.dma_start(sb_wot[:], d_wot[:])
            sb_encTb = sb_encT
            sb_decTb = sb_decT



            # ---- uhT[k, s] + wqT[k, t] (+bq), chunk-interleaved so the
            # first tanh batch only waits for the kc=0 chunks ----
            sum_dt = bf16 if SUMS_BF16 else f32
            sb_uh = consts.tile([P, NH * S], sum_dt)
            sb_wqb = consts.tile([P, NH * TC], f32)
            from concourse.tile import add_dep_helper

            def do_proj(kcs):
                for kc in kcs:
                    ps = psum_big.tile([P, S], f32, tag="big", name="psb")
                    for hc in range(NH):
                        mm = nc.tensor.matmul(
                            ps[:],
                            sb_wct[:, hc * H + kc * P: hc * H + (kc + 1) * P],
                            sb_encT[:, hc * S:(hc + 1) * S],
                            start=(hc == 0), stop=(hc == NH - 1),
                        )
                        if kc == 0 and hc == 0:
                            add_dep_helper(
                                mm.ins, warm_insts[-1].ins, sync=False,
                                reason="warmup before first uh matmul")
                    nc.vector.tensor_copy(sb_uh[:, kc * S:(kc + 1) * S], ps[:])
                    psq = psum_sm.tile([P, TC], f32, tag="sm", name="ps")
                    for hc in range(NH):
                        nc.tensor.matmul(
                            psq[:],
                            sb_wqt[:, hc * H + kc * P: hc * H + (kc + 1) * P],
                            sb_decT[:, hc * TC:(hc + 1) * TC],
                            start=(hc == 0), stop=(hc == NH - 1),
                        )
                    nc.vector.tensor_scalar_add(
                        sb_wqb[:, kc * TC:(kc + 1) * TC], psq[:],
                        sb_bqb[:, kc:kc + 1])

            do_proj([0])

            ps_scores = [
                psum_sc.tile([32, S], f32, tag="scores0", name="scores0"),
                psum_sc.tile([32, S], f32, tag="scores1", name="scores1"),
            ]

            def do_stripe(half, phases=range(NH)):
                h0 = half * 32
                for hc in phases:
                    # batch sizes tuned per phase: wide (16) batches in
                    # the interior phases amortize the per-ACTIVATE
                    # overhead; narrow (8/4) batches at the ramp-up and
                    # the stripe end keep the pipeline fine-grained where
                    # it matters
                    if hc == NH - 1:
                        batches = [(0, 8), (8, 8), (16, 8), (24, 4), (28, 4)]
                    elif hc == 0 and half == 0:
                        batches = [(0, 4), (4, 4), (8, 8), (16, 8), (24, 8)]
                    elif hc == 0:
                        batches = [(0, 8), (8, 8), (16, 8), (24, 8)]
                    else:
                        batches = [(0, 16), (16, 16)]
                    for bt0, bn in batches:
                        sum_t = sums.tile([P, bn * S], sum_dt, tag="sum",
                                          name="sum_t")
                        for tt in range(bn):
                            t = h0 + bt0 + tt
                            nc.vector.tensor_scalar_add(
                                sum_t[:, tt * S:(tt + 1) * S],
                                sb_uh[:, hc * S:(hc + 1) * S],
                                sb_wqb[:, hc * TC + t: hc * TC + t + 1])
                        tanh_t = tanhs.tile([P, bn * S], bf16, tag="tanh",
                                            name="tanh_t")
                        nc.scalar.activation(tanh_t[:], sum_t[:], AF.Tanh)
                        for tt in range(bn):
                            t = h0 + bt0 + tt
                            tl = t - h0
                            lhsT = sb_V[:, hc * 64 + 32 - tl: hc * 64 + 64 - tl]
                            rhs = tanh_t[:, tt * S:(tt + 1) * S]
                            nc.tensor.matmul(
                                ps_scores[half][:, :], lhsT, rhs,
                                start=(hc == 0 and bt0 == 0 and tt == 0),
                                stop=(hc == NH - 1 and bt0 + tt == 32 - 1),
                                skip_group_check=True,
                            )

            # ---- prefolded output projection (emitted after stripe 0 so
            # its DVE work lands once the loop has built up a cushion) ----
            sb_WET = consts.tile([P, NS * H], bf16)
            sb_base = consts.tile([P, NH * TC], bf16)

            def do_prefold_wet(scs):
                # WET[s, k] = sum_h enc[s, h] * Wo[k, h]
                for sc in scs:
                    ps = psum_big.tile([P, H], f32, tag="big", name="psw")
                    for hc in range(NH):
                        nc.tensor.matmul(
                            ps[:],
                            sb_encTb[:, hc * S + sc * P: hc * S + (sc + 1) * P],
                            sb_wot[:, hc * H:(hc + 1) * H],
                            start=(hc == 0), stop=(hc == NH - 1),
                        )
                    nc.vector.tensor_copy(sb_WET[:, sc * H:(sc + 1) * H], ps[:])

            def do_prefold_base():
                # base[k, t] = sum_h Wo[k, H + h] * dec[t, h] + bo[k]
                for kc in range(NH):
                    ps = psum_sm.tile([P, TC], f32, tag="sm", name="psd")
                    for dc in range(NH, 2 * NH):
                        nc.tensor.matmul(
                            ps[:],
                            sb_wot[:, dc * H + kc * P: dc * H + (kc + 1) * P],
                            sb_decTb[:, (dc - NH) * TC:(dc - NH + 1) * TC],
                            start=(dc == NH), stop=(dc == 2 * NH - 1),
                        )
                    nc.vector.tensor_scalar_add(
                        sb_base[:, kc * TC:(kc + 1) * TC], ps[:],
                        sb_bob[:, kc:kc + 1])

            sb_aligns = [consts.tile([32, S], f32, name="sb_exp0"),
                         consts.tile([32, S], f32, name="sb_exp1")]
            sb_align_n = [consts.tile([32, S], f32, name="sb_al0"),
                          consts.tile([32, S], f32, name="sb_al1")]
            sb_alignT = consts.tile([P, NS * TC], bf16)
            sb_attn = consts.tile([P, NH * TC], f32)
            d_attn3 = d_attn[:].rearrange("p (k t) -> p k t", k=NH)
            sb_attn3 = sb_attn[:].rearrange("p (k t) -> p k t", k=NH)

            def do_epilogue(half):
                h0 = half * 32
                sb_align = sb_aligns[half]
                psc = ps_scores[half]
                # softmax pieces: exp with -max bias, row sums, 1/sum
                # no max-subtraction needed: |scores| <= ||v||_1 * 1 ~ 21,
                # far below fp32 exp overflow (~88), and softmax is
                # shift-invariant so the result matches the reference
                sumexp = small.tile([32, 1], f32, tag="st", name="sumexp")
                rsum = small.tile([32, 1], f32, tag="st", name="rsum")
                nc.scalar.activation(sb_align[:, :], psc[:, :], AF.Exp,
                                     accum_out=sumexp[:])
                nc.vector.reciprocal(rsum[:], sumexp[:])
                # normalized align rows for the align_vectors output
                nc.vector.tensor_scalar_mul(sb_align_n[half][:, :],
                                            sb_align[:, :], rsum[:])
                nc.sync.dma_start(d_align[h0:h0 + 32, :], sb_align_n[half][:, :])

                # alignT[s, t-half] from the normalized align rows
                for sc in range(NS):
                    pst = psum_sm.tile([P, 32], f32, tag="sm", name="pst")
                    nc.tensor.transpose(
                        pst[:], sb_align_n[half][:, sc * P:(sc + 1) * P],
                        ident[:])
                    nc.vector.tensor_copy(
                        sb_alignT[:, sc * TC + h0: sc * TC + h0 + 32], pst[:])

                # attn_T[k, t] = base[k, t] + sum_s WET[s, k] * alignT[s, t]
                # sc-major accumulation: each transposed alignT chunk feeds
                # all four kc groups immediately (base was pre-accumulated
                # into the psum tiles during the main loop)
                ps_att = att_psums[half]
                for sc in range(NS):
                    for kc in range(NH):
                        nc.tensor.matmul(
                            ps_att[kc][:],
                            sb_WET[:, sc * H + kc * P: sc * H + (kc + 1) * P],
                            sb_alignT[:, sc * TC + h0: sc * TC + h0 + 32],
                            start=False, stop=(sc == NS - 1),
                            skip_group_check=True)
                for kc in range(NH):
                    nc.vector.tensor_copy(
                        sb_attn3[:, kc, h0:h0 + 32], ps_att[kc][:])
                nc.sync.dma_start(d_attn3[:, :, h0:h0 + 32],
                                  sb_attn3[:, :, h0:h0 + 32])

            do_proj([1])
            do_stripe(0, phases=[0])
            do_proj([2])
            do_stripe(0, phases=[1])
            do_proj([3])
            do_stripe(0, phases=[2, 3])
            do_stripe(1, phases=[0])
            do_prefold_wet([0, 1])
            do_stripe(1, phases=[1])
            do_prefold_wet([2, 3])
            do_prefold_base()
            att_psums = {}
            for half in (0, 1):
                h0 = half * 32
                tiles = []
                for kc in range(NH):
                    tag = "big" if kc < 2 else "ps2"
                    ps = psum_sm.tile([P, 32], f32, tag=tag,
                                      name=f"att{half}{kc}") if kc >= 2 else \
                        psum_big.tile([P, 32], f32, tag="big",
                                      name=f"att{half}{kc}")
                    nc.tensor.matmul(
                        ps[:], identB[:, :],
                        sb_base[:, kc * TC + h0: kc * TC + h0 + 32],
                        start=True, stop=False, skip_group_check=True)
                    tiles.append(ps)
                att_psums[half] = tiles
            do_epilogue(0)
            do_stripe(1, phases=[2, 3])
            do_epilogue(1)

    nc.compile()
    return nc


def _get_nc():
    global _cached
    if _cached is None:
        _cached = _build()
    return _cached


def _chunk_cols(a):
    """[n*128, C] -> [128, n*C] with row-chunk i at cols [i*C:(i+1)*C]."""
    n = a.shape[0] // P
    return np.ascontiguousarray(
        a.reshape(n, P, a.shape[1]).transpose(1, 0, 2).reshape(P, -1))


def make_in_maps(dec_output, enc_output, Wq, bq, Wc, v, Wo, bo):
    bf16 = np.float16

    wqt = _chunk_cols(np.ascontiguousarray(Wq.T)).astype(bf16)
    wct = _chunk_cols(np.ascontiguousarray(Wc.T)).astype(bf16)
    wot = _chunk_cols(np.ascontiguousarray(Wo.T)).astype(bf16)
    bqb = np.ascontiguousarray(bq.reshape(NH, P).T)
    bob = np.ascontiguousarray(bo.reshape(NH, P).T)
    # Sliding-window padded v: column (hc*64 + 32) holds v chunk hc; the
    # lhsT slice [hc*64 + 32 - tl : hc*64 + 64 - tl] puts v at window col tl
    V = np.zeros((P, NH, 64), dtype=np.float32)
    for hc in range(NH):
        V[:, hc, 32] = v[hc * P:(hc + 1) * P]
    V = np.ascontiguousarray(V.reshape(P, NH * 64)).astype(bf16)

    in_maps = []
    for c in range(N_CORES):
        b, th = c // 2, c % 2
        t0 = th * TC
        decT = _chunk_cols(
            np.ascontiguousarray(dec_output[b].T[:, t0:t0 + TC])).astype(bf16)
        encT_sb = _chunk_cols(
            np.ascontiguousarray(enc_output[b].T)).astype(bf16)
        in_maps.append({
            "decT": decT, "encT": encT_sb,
            "wqt": wqt, "wct": wct, "wot": wot,
            "V": V, "bqb": bqb, "bob": bob,
        })
    return in_maps


def assemble(results):
    attn_h = np.empty((B, T, H), dtype=np.float32)
    align_vectors = np.empty((T, B, S), dtype=np.float32)
    for c in range(N_CORES):
        b, th = c // 2, c % 2
        t0 = th * TC
        align_vectors[t0:t0 + TC, b, :] = results[c]["align_out"]
        a = results[c]["attn_out"].reshape(P, NH, TC)
        attn_h[b, t0:t0 + TC, :] = a.transpose(2, 1, 0).reshape(TC, H)
    return attn_h, align_vectors


def run(trace=False, **inputs):
    from concourse.bass_utils import run_bass_kernel_spmd

    args = {k: np.asarray(inputs[k], dtype=np.float32)
            for k in ("dec_output", "enc_output", "Wq", "bq", "Wc", "v",
                      "Wo", "bo")}
    nc = _get_nc()
    in_maps = make_in_maps(**args)
    if trace:
        try:
            from antenv.axon_hooks import set_axon_ntff_profile_hook
            from trn_agent_boot.trn_boot import _ntff_profile_via_ctypes
            set_axon_ntff_profile_hook(
                _ntff_profile_via_ctypes("/opt/axon/libaxon_pjrt.so"))
        except Exception:
            pass
    res = run_bass_kernel_spmd(nc, in_maps, core_ids=list(range(N_CORES)),
                               trace=trace)
    out = assemble(res.results)
    return out, res


def kernel(**inputs):
    out, _ = run(trace=False, **inputs)
    return out


# revision 52
# speedup vs baseline: 1.2020x; 1.0044x over previous
"""Trainium2 Bass kernel for Bahdanau (MLP) additive attention.

Reference computation (B=4, T=128, S=512, H=512):
    wq = dec @ Wq.T + bq                    [B,T,H]
    uh = enc @ Wc.T                         [B,S,H]
    scores[b,t,s] = sum_h v[h] * tanh(wq[b,t,h] + uh[b,s,h])
    align = softmax(scores, axis=-1)        [B,T,S]
    c = align @ enc                         [B,T,H]
    attn_h = [c, dec] @ Wo.T + bo           [B,T,H]
    returns (attn_h, align.transpose(1,0,2))

Sharding: pure data parallel over (batch, T-half) -> 8 cores, 64 queries
per core, no cross-core communication.  The dominant cost is tanh over
B*T*S*H = 134M elements: 16.8M per core on the scalar (ACT) engine at
1 elem/lane/cycle -> ~110 us busy, which this kernel keeps ~97% fed.

Per-core dataflow (hidden index on partitions everywhere):
    PE:  uhT = Wc.T-chunks @ encT, wqT = Wq.T-chunks @ decT (+bq), fp16
         inputs (fp16 rounding ~5e-4, 4x cheaper DMA + FWL weight loads)
    DVE: sum(t,hc) = uhT[hc] + wqT[hc, t]: tensor_scalar add with the
         per-partition scalar taken from wqT column t; fp16 in/out hits
         the DVE 4x perf mode (~270 ns per [128,512] slot)
    ACT: tanh over batched tiles (t-slots sharing one uh chunk), fp16
         out; batch width is 16 slots in interior phases (amortizing the
         352-cycle ACTIVATE overhead) and 8/4 at ramps and stripe ends
    PE:  scores[stripe] += V_window(t).T @ tanh(t,hc), f32r, N=512.
         V_window: a 32-wide sliding slice of a zero-padded buffer whose
         column t holds v_chunk - an M=32 matmul accumulates row t of the
         32-row stripe and adds zero to the other rows, working around
         the 32-aligned PE column-group constraint.
    Scores accumulate in two independent 32-row psum stripes so stripe
    0's softmax/output work overlaps stripe 1's main loop.
    softmax: exp straight off the scores psum (|scores| <= ||v||_1 ~ 21
         << 88, and softmax is shift-invariant, so no max subtraction)
         with fused row sums (accum_out), then DVE reciprocal + scale.
    out projection, prefolded in the prologue (PE, fp16):
         WET[s,k]  = sum_h enc[s,h] Wo[k,h]
         base[k,t] = sum_h Wo[k,H+h] dec[t,h] + bo  (pre-accumulated into
         the attn psum tiles via an identity matmul during the main loop)
    epilogue per stripe: alignT = transpose(align) on PE, then
         attn_T[k,t] = base + sum_s WET[s,k] alignT[s,t], copied out.

Further scheduling tricks: input DMAs split across the three DMA queues
(sync/scalar HWDGE + gpsimd SWDGE); PE warmed up with dummy matmuls so
the prologue matmuls run at 2.4 GHz; uh/wq chunks and the prefold are
emitted between loop phases so their PE/DVE work fills idle slots; the
first/last tanh batches are halved to shorten the pipeline ramps.

Measured on trn2 (8 cores): ~145 us HW exec, rel err ~7e-4 vs the fp32
jax reference.
"""

import numpy as np

B, T, S, H = 4, 128, 512, 512
P = 128
NH = H // P          # 4 h-chunks
NS = S // P          # 4 s-chunks
TC = 64              # queries per core
GB = 8               # t-slots per ACT batch (one uh chunk shared)
FD = GB * S          # 4096 free dim of the batched tanh tile
NG = 32 // GB        # groups per (stripe, hc) phase
N_CORES = 8

SCORES_MODE = "f32r"
SUMS_BF16 = True

_cached = None


def _build():
    import concourse.bacc as bacc
    import concourse.tile as tile
    import concourse.mybir as mybir
    from concourse.masks import make_identity

    f32 = mybir.dt.float32
    bf16 = mybir.dt.float16  # fp16: same speed, 4x finer rounding than bfloat16
    f32r = mybir.dt.float32r
    AF = mybir.ActivationFunctionType

    nc = bacc.Bacc("TRN2", target_bir_lowering=False, debug=False,
                   num_devices=N_CORES)

    d_decT = nc.dram_tensor("decT", [P, NH * TC], bf16, kind="ExternalInput")
    d_wqt = nc.dram_tensor("wqt", [P, NH * H], bf16, kind="ExternalInput")
    d_wct = nc.dram_tensor("wct", [P, NH * H], bf16, kind="ExternalInput")
    d_encT = nc.dram_tensor("encT", [P, NH * S], bf16, kind="ExternalInput")
    d_V = nc.dram_tensor("V", [P, NH * 64], bf16, kind="ExternalInput")
    d_bqb = nc.dram_tensor("bqb", [P, NH], f32, kind="ExternalInput")
    d_bob = nc.dram_tensor("bob", [P, NH], f32, kind="ExternalInput")
    d_wot = nc.dram_tensor("wot", [P, 2 * NH * H], bf16, kind="ExternalInput")

    d_align = nc.dram_tensor("align_out", [TC, S], f32, kind="ExternalOutput")
    d_attn = nc.dram_tensor("attn_out", [P, NH * TC], f32, kind="ExternalOutput")

    with tile.TileContext(nc) as tc:
        with (
            tc.tile_pool(name="consts", bufs=1) as consts,
            tc.tile_pool(name="sums", bufs=4) as sums,
            tc.tile_pool(name="tanhs", bufs=4) as tanhs,
            tc.tile_pool(name="small", bufs=4) as small,
            tc.tile_pool(name="psc", bufs=1, space="PSUM") as psum_sc,
            tc.tile_pool(name="pbig", bufs=2, space="PSUM") as psum_big,
            tc.tile_pool(name="psm", bufs=2, space="PSUM") as psum_sm,
        ):
            # ---- input DMAs, critical-path tensors first ----
            ident = consts.tile([32, 32], f32)
            make_identity(nc, ident[:, :])
            identB = consts.tile([P, P], bf16)
            make_identity(nc, identB[:, :])
            junk = consts.tile([P, S], bf16)
            nc.vector.memset(junk[:], 0.5)
            ps_warm = psum_sm.tile([P, S], f32, tag="sm", name="ps_warm")
            warm_insts = []
            for i in range(20):
                warm_insts.append(nc.tensor.matmul(
                    ps_warm[:], junk[:, :P], junk[:],
                    start=(i == 0), stop=(i == 19), skip_group_check=True))

            # three parallel DMA queues (sync/scalar HWDGE + gpsimd SWDGE),
            # each ~110 GB/s; the uh/wq inputs are split so every queue
            # carries ~0.5 MB of the critical path
            sb_encT = consts.tile([P, NH * S], bf16)
            sb_wct = consts.tile([P, NH * H], bf16)
            nc.sync.dma_start(sb_encT[:], d_encT[:])
            nc.scalar.dma_start(sb_wct[:], d_wct[:])
            sb_wqt = consts.tile([P, NH * H], bf16)
            nc.gpsimd.dma_start(sb_wqt[:], d_wqt[:])
            sb_decT = consts.tile([P, NH * TC], bf16)
            nc.sync.dma_start(sb_decT[:], d_decT[:])
            sb_V = consts.tile([P, NH * 64], bf16)
            nc.sync.dma_start(sb_V[:], d_V[:])
            sb_bqb = consts.tile([P, NH], f32)
            nc.sync.dma_start(sb_bqb[:], d_bqb[:])
            sb_bob = consts.tile([P, NH], f32)
            nc.scalar.dma_start(sb_bob[:], d_bob[:])
            sb_wot = consts.tile([P, 2 * NH * H], bf16)
            nc.sync.dma_start(sb_wot[:], d_wot[:])
            sb_encTb = sb_encT
            sb_decTb = sb_decT



            # ---- uhT[k, s] + wqT[k, t] (+bq), chunk-interleaved so the
            # first tanh batch only waits for the kc=0 chunks ----
            sum_dt = bf16 if SUMS_BF16 else f32
            sb_uh = consts.tile([P, NH * S], sum_dt)
            sb_wqb = consts.tile([P, NH * TC], f32)
            from concourse.tile import add_dep_helper

            def do_proj(kcs):
                for kc in kcs:
                    ps = psum_big.tile([P, S], f32, tag="big", name="psb")
                    for hc in range(NH):
                        mm = nc.tensor.matmul(
                            ps[:],
                            sb_wct[:, hc * H + kc * P: hc * H + (kc + 1) * P],
                            sb_encT[:, hc * S:(hc + 1) * S],
                            start=(hc == 0), stop=(hc == NH - 1),
                        )
                        if kc == 0 and hc == 0:
                            add_dep_helper(
                                mm.ins, warm_insts[-1].ins, sync=False,
                                reason="warmup before first uh matmul")
                    nc.vector.tensor_copy(sb_uh[:, kc * S:(kc + 1) * S], ps[:])
                    psq = psum_sm.tile([P, TC], f32, tag="sm", name="ps")
                    for hc in range(NH):
                        nc.tensor.matmul(
                            psq[:],
                            sb_wqt[:, hc * H + kc * P: hc * H + (kc + 1) * P],
                            sb_decT[:, hc * TC:(hc + 1) * TC],
                            start=(hc == 0), stop=(hc == NH - 1),
                        )
                    nc.vector.tensor_scalar_add(
                        sb_wqb[:, kc * TC:(kc + 1) * TC], psq[:],
                        sb_bqb[:, kc:kc + 1])

            do_proj([0])

            ps_scores = [
                psum_sc.tile([32, S], f32, tag="scores0", name="scores0"),
                psum_sc.tile([32, S], f32, tag="scores1", name="scores1"),
            ]

            def do_stripe(half, phases=range(NH)):
                h0 = half * 32
                for hc in phases:
                    # batch sizes tuned per phase: wide (16) batches in
                    # the interior phases amortize the per-ACTIVATE
                    # overhead; narrow (8/4) batches at the ramp-up and
                    # the stripe end keep the pipeline fine-grained where
                    # it matters
                    if hc == NH - 1:
                        batches = [(0, 16), (16, 8), (24, 4), (28, 4)]
                    elif hc == 0 and half == 0:
                        batches = [(0, 4), (4, 4), (8, 8), (16, 16)]
                    else:
                        batches = [(0, 16), (16, 16)]
                    for bt0, bn in batches:
                        sum_t = sums.tile([P, bn * S], sum_dt, tag="sum",
                                          name="sum_t")
                        for tt in range(bn):
                            t = h0 + bt0 + tt
                            nc.vector.tensor_scalar_add(
                                sum_t[:, tt * S:(tt + 1) * S],
                                sb_uh[:, hc * S:(hc + 1) * S],
                                sb_wqb[:, hc * TC + t: hc * TC + t + 1])
                        tanh_t = tanhs.tile([P, bn * S], bf16, tag="tanh",
                                            name="tanh_t")
                        nc.scalar.activation(tanh_t[:], sum_t[:], AF.Tanh)
                        for tt in range(bn):
                            t = h0 + bt0 + tt
                            tl = t - h0
                            lhsT = sb_V[:, hc * 64 + 32 - tl: hc * 64 + 64 - tl]
                            rhs = tanh_t[:, tt * S:(tt + 1) * S]
                            nc.tensor.matmul(
                                ps_scores[half][:, :], lhsT, rhs,
                                start=(hc == 0 and bt0 == 0 and tt == 0),
                                stop=(hc == NH - 1 and bt0 + tt == 32 - 1),
                                skip_group_check=True,
                            )

            # ---- prefolded output projection (emitted after stripe 0 so
            # its DVE work lands once the loop has built up a cushion) ----
            sb_WET = consts.tile([P, NS * H], bf16)
            sb_base = consts.tile([P, NH * TC], bf16)

            def do_prefold_wet(scs):
                # WET[s, k] = sum_h enc[s, h] * Wo[k, h]
                for sc in scs:
                    ps = psum_big.tile([P, H], f32, tag="big", name="psw")
                    for hc in range(NH):
                        nc.tensor.matmul(
                            ps[:],
                            sb_encTb[:, hc * S + sc * P: hc * S + (sc + 1) * P],
                            sb_wot[:, hc * H:(hc + 1) * H],
                            start=(hc == 0), stop=(hc == NH - 1),
                        )
                    nc.vector.tensor_copy(sb_WET[:, sc * H:(sc + 1) * H], ps[:])

            def do_prefold_base():
                # base[k, t] = sum_h Wo[k, H + h] * dec[t, h] + bo[k]
                for kc in range(NH):
                    ps = psum_sm.tile([P, TC], f32, tag="sm", name="psd")
                    for dc in range(NH, 2 * NH):
                        nc.tensor.matmul(
                            ps[:],
                            sb_wot[:, dc * H + kc * P: dc * H + (kc + 1) * P],
                            sb_decTb[:, (dc - NH) * TC:(dc - NH + 1) * TC],
                            start=(dc == NH), stop=(dc == 2 * NH - 1),
                        )
                    nc.vector.tensor_scalar_add(
                        sb_base[:, kc * TC:(kc + 1) * TC], ps[:],
                        sb_bob[:, kc:kc + 1])

            sb_aligns = [consts.tile([32, S], f32, name="sb_exp0"),
                         consts.tile([32, S], f32, name="sb_exp1")]
            sb_align_n = [consts.tile([32, S], f32, name="sb_al0"),
                          consts.tile([32, S], f32, name="sb_al1")]
            sb_alignT = consts.tile([P, NS * TC], bf16)
            sb_attn = consts.tile([P, NH * TC], f32)
            d_attn3 = d_attn[:].rearrange("p (k t) -> p k t", k=NH)
            sb_attn3 = sb_attn[:].rearrange("p (k t) -> p k t", k=NH)

            def do_epilogue(half):
                h0 = half * 32
                sb_align = sb_aligns[half]
                psc = ps_scores[half]
                # softmax pieces: exp with -max bias, row sums, 1/sum
                # no max-subtraction needed: |scores| <= ||v||_1 * 1 ~ 21,
                # far below fp32 exp overflow (~88), and softmax is
                # shift-invariant so the result matches the reference
                sumexp = small.tile([32, 1], f32, tag="st", name="sumexp")
                rsum = small.tile([32, 1], f32, tag="st", name="rsum")
                nc.scalar.activation(sb_align[:, :], psc[:, :], AF.Exp,
                                     accum_out=sumexp[:])
                nc.vector.reciprocal(rsum[:], sumexp[:])
                # normalized align rows for the align_vectors output
                nc.vector.tensor_scalar_mul(sb_align_n[half][:, :],
                                            sb_align[:, :], rsum[:])
                nc.sync.dma_start(d_align[h0:h0 + 32, :], sb_align_n[half][:, :])

                # alignT[s, t-half] from the normalized align rows
                for sc in range(NS):
                    pst = psum_sm.tile([P, 32], f32, tag="sm", name="pst")
                    nc.tensor.transpose(
                        pst[:], sb_align_n[half][:, sc * P:(sc + 1) * P],
                        ident[:])
                    nc.vector.tensor_copy(
                        sb_alignT[:, sc * TC + h0: sc * TC + h0 + 32], pst[:])

                # attn_T[k, t] = base[k, t] + sum_s WET[s, k] * alignT[s, t]
                # sc-major accumulation: each transposed alignT chunk feeds
                # all four kc groups immediately (base was pre-accumulated
                # into the psum tiles during the main loop)
                ps_att = att_psums[half]
                for sc in range(NS):
                    for kc in range(NH):
                        nc.tensor.matmul(
                            ps_att[kc][:],
                            sb_WET[:, sc * H + kc * P: sc * H + (kc + 1) * P],
                            sb_alignT[:, sc * TC + h0: sc * TC + h0 + 32],
                            start=False, stop=(sc == NS - 1),
                            skip_group_check=True)
                for kc in range(NH):
                    nc.vector.tensor_copy(
                        sb_attn3[:, kc, h0:h0 + 32], ps_att[kc][:])
                nc.sync.dma_start(d_attn3[:, :, h0:h0 + 32],
                                  sb_attn3[:, :, h0:h0 + 32])

            do_proj([1])
            do_stripe(0, phases=[0])
            do_proj([2])
            do_stripe(0, phases=[1])
            do_proj([3])
            do_stripe(0, phases=[2, 3])
            do_stripe(1, phases=[0])
            do_prefold_wet([0, 1])
            do_stripe(1, phases=[1])
            do_prefold_wet([2, 3])
            do_prefold_base()
            att_psums = {}
            for half in (0, 1):
                h0 = half * 32
                tiles = []
                for kc in range(NH):
                    tag = "big" if kc < 2 else "ps2"
                    ps = psum_sm.tile([P, 32], f32, tag=tag,
                                      name=f"att{half}{kc}") if kc >= 2 else \
                        psum_big.tile([P, 32], f32, tag="big",
                                      name=f"att{half}{kc}")
                    nc.tensor.matmul(
                        ps[:], identB[:, :],
                        sb_base[:, kc * TC + h0: kc * TC + h0 + 32],
                        start=True, stop=False, skip_group_check=True)
                    tiles.append(ps)
                att_psums[half] = tiles
            do_epilogue(0)
            do_stripe(1, phases=[2, 3])
            do_epilogue(1)

    nc.compile()
    return nc


def _get_nc():
    global _cached
    if _cached is None:
        _cached = _build()
    return _cached


def _chunk_cols(a):
    """[n*128, C] -> [128, n*C] with row-chunk i at cols [i*C:(i+1)*C]."""
    n = a.shape[0] // P
    return np.ascontiguousarray(
        a.reshape(n, P, a.shape[1]).transpose(1, 0, 2).reshape(P, -1))


def make_in_maps(dec_output, enc_output, Wq, bq, Wc, v, Wo, bo):
    bf16 = np.float16

    wqt = _chunk_cols(np.ascontiguousarray(Wq.T)).astype(bf16)
    wct = _chunk_cols(np.ascontiguousarray(Wc.T)).astype(bf16)
    wot = _chunk_cols(np.ascontiguousarray(Wo.T)).astype(bf16)
    bqb = np.ascontiguousarray(bq.reshape(NH, P).T)
    bob = np.ascontiguousarray(bo.reshape(NH, P).T)
    # Sliding-window padded v: column (hc*64 + 32) holds v chunk hc; the
    # lhsT slice [hc*64 + 32 - tl : hc*64 + 64 - tl] puts v at window col tl
    V = np.zeros((P, NH, 64), dtype=np.float32)
    for hc in range(NH):
        V[:, hc, 32] = v[hc * P:(hc + 1) * P]
    V = np.ascontiguousarray(V.reshape(P, NH * 64)).astype(bf16)

    in_maps = []
    for c in range(N_CORES):
        b, th = c // 2, c % 2
        t0 = th * TC
        decT = _chunk_cols(
            np.ascontiguousarray(dec_output[b].T[:, t0:t0 + TC])).astype(bf16)
        encT_sb = _chunk_cols(
            np.ascontiguousarray(enc_output[b].T)).astype(bf16)
        in_maps.append({
            "decT": decT, "encT": encT_sb,
            "wqt": wqt, "wct": wct, "wot": wot,
            "V": V, "bqb": bqb, "bob": bob,
        })
    return in_maps


def assemble(results):
    attn_h = np.empty((B, T, H), dtype=np.float32)
    align_vectors = np.empty((T, B, S), dtype=np.float32)
    for c in range(N_CORES):
        b, th = c // 2, c % 2
        t0 = th * TC
        align_vectors[t0:t0 + TC, b, :] = results[c]["align_out"]
        a = results[c]["attn_out"].reshape(P, NH, TC)
        attn_h[b, t0:t0 + TC, :] = a.transpose(2, 1, 0).reshape(TC, H)
    return attn_h, align_vectors


def run(trace=False, **inputs):
    from concourse.bass_utils import run_bass_kernel_spmd

    args = {k: np.asarray(inputs[k], dtype=np.float32)
            for k in ("dec_output", "enc_output", "Wq", "bq", "Wc", "v",
                      "Wo", "bo")}
    nc = _get_nc()
    in_maps = make_in_maps(**args)
    if trace:
        try:
            from antenv.axon_hooks import set_axon_ntff_profile_hook
            from trn_agent_boot.trn_boot import _ntff_profile_via_ctypes
            set_axon_ntff_profile_hook(
                _ntff_profile_via_ctypes("/opt/axon/libaxon_pjrt.so"))
        except Exception:
            pass
    res = run_bass_kernel_spmd(nc, in_maps, core_ids=list(range(N_CORES)),
                               trace=trace)
    out = assemble(res.results)
    return out, res


def kernel(**inputs):
    out, _ = run(trace=False, **inputs)
    return out
